# revision 20
# baseline (speedup 1.0000x reference)
"""Bass/Trainium2 kernel for nn_EquivSetGNN3 (gnn_message_passing).

Math (reference): x = relu(x@W_in+b_in); x0 = x
  2 layers of: Xe = segsum_E((x@W1+b1)[V]); Xev = cat(x[V], Xe[E])@W2+b2
               Xv = segsum_V(Xev); x = relu((0.5*Xv + 0.5*x0)@W3 + b3)

Algebraic restructuring (avoids all [nnz, C] feature materialization):
  Xe = (segsum_E x[V]) @ W1 + deg_E (x) b1
  Xv = deg_V (*) (x @ W2a) + (segsum_V Xe[E]) @ W2b + deg_V (x) b2
where W2a = W2[:C], W2b = W2[C:].

Distribution: phase A is flipped to avoid AllGathering x. Each core keeps
only its own x slice and processes the incidences whose node lives in
that slice (gather indices are local, < 6250), producing W1-transformed
PARTIAL Xe rows for ALL 25000 edges; one AllReduce(add) per layer then
sums and replicates Xe. The b1*degE bias is linear, so it is folded
host-side into the phase-B restart tensor (X0ADD includes
(W2b^T b1) * s2[v], s2[v] = sum of degE over v's edges). Phase B gathers
Xe rows for its own nodes (single int16 bucket) and writes only the
local x slice — no x collective exists at all.

Segment sums run as dma_gather of 256B f16 rows + one-hot matmuls on the
TensorEngine. Gathers are batched into grouped-superchunk calls split at
~2048 idxs across 4 SWDGE queues; the one-hot P matrices are generated on
the fly by the DVE (is_equal vs an iota row, broadcast APs). Relu+bias
runs on the DVE so the Scalar engine stays free.
"""
import numpy as np

import concourse.bacc as bacc
import concourse.mybir as mybir
import concourse.tile as tile
from concourse.bass_utils import run_bass_kernel_spmd

f32 = mybir.dt.float32
f16 = mybir.dt.float16
i16 = mybir.dt.int16

N = 50000
M = 25000
NNZ = 800000
C = 128
R = 8
NO = N // R          # 6250 nodes per core
EO = M // R          # 3125 edges per core
SCP = (M + 127) // 128    # 196 global edge superchunks (phase A partials)
SCB = (NO + 127) // 128   # 49 node superchunks per core
GP = 12              # phase-A global edge superchunks per gather group
GB = 4               # phase-B (and prologue) superchunks per group
N_LAYERS = 2
ALPHA = 0.5
SPLIT_CHUNKS = 16    # max chunks (128 idxs each) per dma_gather call

GROUPS_P = [list(range(g, min(g + GP, SCP))) for g in range(0, SCP, GP)]
GROUPS_B = [list(range(g, min(g + GB, SCB))) for g in range(0, SCB, GB)]
assert NO < 32768 and EO * R < 32768  # int16 gather indices

_cache = {}


def _wrap_idx(flat):
    """[L] int -> [128, L//16] int16 (idx i at partition i%16, col i//16;
    replicated 8x across partition groups for the 8 gpsimd cores)."""
    w = flat.reshape(-1, 16).T.astype(np.int16)
    return np.ascontiguousarray(np.tile(w, (8, 1)))


def _wrap_rel(flat):
    """[L] int -> [128, L//128] f16 (value i at partition i%128, col i//128)."""
    return np.ascontiguousarray(flat.reshape(-1, 128).T.astype(np.float16))


def _pad_to(arr, n, val):
    out = np.full(n, val, dtype=arr.dtype)
    out[: len(arr)] = arr
    return out


def _prepare(V, E):
    """Host-side preprocessing: sorted/sharded/padded gather index+rel
    arrays (see module docstring)."""
    # ---- phase A: per core, incidences with v in the core's slice,
    # sorted by global edge id; partial Xe over ALL edges ----
    nchP = np.zeros(SCP, np.int64)
    winP = []
    perA = []
    for r in range(R):
        m = (V // NO) == r
        Vr, Er = V[m], E[m]
        o = np.argsort(Er, kind="stable")
        Vr, Er = Vr[o], Er[o]
        lo = np.searchsorted(Er, np.arange(SCP) * 128)
        hi = np.searchsorted(Er, np.minimum(np.arange(SCP) * 128 + 128, M))
        nchP = np.maximum(nchP, -(-(hi - lo) // 128))
        winP.append((Vr % NO, Er % 128, lo, hi))
    assert (nchP > 0).all()

    # ---- phase B: incidences sorted by V (node-major); gathers Xe[E] ----
    oB = np.argsort(V, kind="stable")
    Vb, Eb = V[oB], E[oB]
    v0 = np.arange(R)[:, None] * NO + np.arange(SCB)[None, :] * 128
    v1 = np.minimum(v0 + 128, (np.arange(R)[:, None] + 1) * NO)
    lo2 = np.searchsorted(Vb, v0.ravel()).reshape(R, SCB)
    hi2 = np.searchsorted(Vb, v1.ravel()).reshape(R, SCB)
    cnt2 = hi2 - lo2
    nchB = (-(-cnt2.max(0) // 128)).astype(np.int64)
    assert (nchB > 0).all()

    meta = {"nchP": nchP.tolist(), "nchB": nchB.tolist()}

    degE = np.bincount(E, minlength=M).astype(np.float32)
    degV = np.bincount(V, minlength=N).astype(np.float32)
    s2 = np.bincount(V, weights=degE[E], minlength=N).astype(np.float32)

    relVb = Vb % NO - (Vb % NO // 128 * 128)   # node offset within SC

    per_core = []
    for r in range(R):
        vloc, erel, lo, hi = winP[r]
        idxA_parts, relA_parts = [], []
        for grp in GROUPS_P:
            for t in grp:
                la = int(nchP[t]) * 128
                idxA_parts.append(_pad_to(vloc[lo[t] : hi[t]], la, 0))
                relA_parts.append(_pad_to(erel[lo[t] : hi[t]], la, -1))
        idxA = np.concatenate(idxA_parts)
        relA = np.concatenate(relA_parts)

        idxB_parts, relB_parts = [], []
        for grp in GROUPS_B:
            for s in grp:
                lb = int(nchB[s]) * 128
                idxB_parts.append(_pad_to(Eb[lo2[r, s] : hi2[r, s]], lb, 0))
                relB_parts.append(_pad_to(relVb[lo2[r, s] : hi2[r, s]], lb, -1))
        idxB = np.concatenate(idxB_parts)
        relB = np.concatenate(relB_parts)

        per_core.append(
            {
                "idxA": _wrap_idx(idxA),
                "relA": _wrap_rel(relA),
                "idxB": _wrap_idx(idxB),
                "relB": _wrap_rel(relB),
                "degV": degV[r * NO : (r + 1) * NO],
                "s2": s2[r * NO : (r + 1) * NO],
            }
        )
    return meta, per_core


def _build(meta):
    nchP = meta["nchP"]   # [SCP]
    nchB = meta["nchB"]   # [SCB]
    LA = sum(nchP) * 128
    LB = sum(nchB) * 128
    NCHA_TOT = LA // 128
    NCHB_TOT = LB // 128

    nc = bacc.Bacc("TRN2", target_bir_lowering=False, debug=False, num_devices=R,
                   num_swdge_queues=4)

    # ---- kernel I/O ----
    xsh = nc.declare_dram_parameter("xsh", [NO, C], f32, isOutput=False)
    w_in = nc.declare_dram_parameter("w_in", [C, C], f32, isOutput=False)
    w1 = nc.declare_dram_parameter("w1", [C, C], f32, isOutput=False)
    w2a = nc.declare_dram_parameter("w2a", [C, C], f32, isOutput=False)
    w2b = nc.declare_dram_parameter("w2b", [C, C], f32, isOutput=False)
    w3h = nc.declare_dram_parameter("w3h", [C, C], f32, isOutput=False)
    b_in = nc.declare_dram_parameter("b_in", [C, 1], f32, isOutput=False)
    b3d = nc.declare_dram_parameter("b3", [C, 1], f32, isOutput=False)
    x0add_d = nc.declare_dram_parameter("x0add", [C, NO], f32, isOutput=False)
    dvrep_d = nc.declare_dram_parameter("dvrep", [C, NO], f32, isOutput=False)
    idxA_d = nc.declare_dram_parameter("idxA", [128, LA // 16], i16, isOutput=False)
    relA_d = nc.declare_dram_parameter("relA", [128, NCHA_TOT], f16, isOutput=False)
    idxB_d = nc.declare_dram_parameter("idxB", [128, LB // 16], i16, isOutput=False)
    relB_d = nc.declare_dram_parameter("relB", [128, NCHB_TOT], f16, isOutput=False)
    xout = nc.declare_dram_parameter("xout", [NO, C], f32, isOutput=True)

    # ---- internal DRAM ----
    agx = [nc.dram_tensor(f"agx_{l}", [NO, C], f16) for l in range(N_LAYERS)]
    xep = [nc.dram_tensor(f"xep_{l}", [M, C], f16) for l in range(N_LAYERS)]
    xe_full = [nc.dram_tensor(f"xe_full_{l}", [M, C], f16, addr_space="Shared")
               for l in range(N_LAYERS)]

    rg = [list(range(R))]
    qrr = [0]

    def next_q():
        q = qrr[0]
        qrr[0] = (q + 1) % 4
        return q

    with tile.TileContext(nc) as tc:
        with (
            tc.tile_pool(name="const", bufs=1) as cp,
            tc.tile_pool(name="work", bufs=2) as wp,
            tc.tile_pool(name="ptiles", bufs=3) as pp,
            tc.tile_pool(name="psA", bufs=2, space="PSUM") as psA,
            tc.tile_pool(name="psB", bufs=2, space="PSUM") as psB,
            tc.tile_pool(name="psC", bufs=2, space="PSUM") as psC,
            tc.tile_pool(name="psD", bufs=2, space="PSUM") as psD,
        ):
            # ---------- persistent tiles ----------
            W_IN = cp.tile([C, C], f32)
            W1 = cp.tile([C, C], f32)
            W2A = cp.tile([C, C], f32)
            W2B = cp.tile([C, C], f32)
            W3H = cp.tile([C, C], f32)
            BIN = cp.tile([C, 1], f32)
            B3 = cp.tile([C, 1], f32)
            IDXA = cp.tile([128, LA // 16], i16)
            IDXB = cp.tile([128, LB // 16], i16)
            RELA = cp.tile([128, NCHA_TOT], f16)
            RELB = cp.tile([128, NCHB_TOT], f16)
            XFM = cp.tile([C, NO], f32)
            X0B = cp.tile([C, NO], f32)
            IOTAF = cp.tile([128, 128], f32)
            IOTA16 = cp.tile([128, 128], f16)
            PIDX = cp.tile([128, 1], f32)
            IDENT = cp.tile([128, 128], f32)

            for t, d in [
                (W_IN, w_in), (W1, w1), (W2A, w2a), (W2B, w2b), (W3H, w3h),
                (BIN, b_in), (B3, b3d),
                (IDXA, idxA_d), (IDXB, idxB_d), (RELA, relA_d), (RELB, relB_d),
            ]:
                nc.sync.dma_start(t[:], d[:])

            nc.gpsimd.iota(IOTAF[:], [[1, 128]], channel_multiplier=0,
                           allow_small_or_imprecise_dtypes=True)
            nc.gpsimd.iota(PIDX[:], [[1, 1]], channel_multiplier=1,
                           allow_small_or_imprecise_dtypes=True)
            nc.vector.tensor_scalar(IDENT[:], IOTAF[:], PIDX[:], None,
                                    mybir.AluOpType.is_equal)
            nc.vector.tensor_copy(IOTA16[:], IOTAF[:])

            def split_gather(gt, col0, nch_tot, src_ap, idx_tile, slot0):
                """One logical gather as ceil(nch_tot/SPLIT_CHUNKS) calls on
                rotating SWDGE queues (parallel descriptor gen)."""
                done = 0
                while done < nch_tot:
                    step = min(SPLIT_CHUNKS, nch_tot - done)
                    sl = slot0 + done * 128
                    nc.gpsimd.dma_gather(
                        out_ap=gt[:, col0 + done : col0 + done + step, :],
                        in_ap=src_ap,
                        idxs_ap=idx_tile[:, sl // 16 : (sl + step * 128) // 16],
                        num_idxs=step * 128, num_idxs_reg=step * 128,
                        elem_size=C, single_packet=False, queue_num=next_q(),
                    )
                    done += step

            def pgen(rel_tile, c0, nch):
                """P[p, c, j] = (rel[p, c0+c] == j), one DVE op."""
                P = pp.tile([128, max(nch, 1), 128], f16, tag="P")
                nc.vector.tensor_tensor(
                    P[:, :nch, :],
                    rel_tile[:, c0 : c0 + nch].unsqueeze(2)
                        .broadcast_to([128, nch, 128]),
                    IOTA16[:].unsqueeze(1).broadcast_to([128, nch, 128]),
                    mybir.AluOpType.is_equal)
                return P

            def emit_rm(src_fm, fm0, nn, ptr, xrm, dstt, d0):
                """Transpose feature-major [C, nn] slice (cols fm0..) to
                row-major blocks and DMA to DRAM rows [d0, d0+nn)."""
                for si in range((nn + 127) // 128):
                    ns = min(128, nn - si * 128)
                    blk = slice(si * 128, si * 128 + C)
                    nc.tensor.transpose(
                        ptr[:ns, blk],
                        src_fm[:, fm0 + si * 128 : fm0 + si * 128 + ns],
                        IDENT[:])
                    nc.vector.tensor_copy(xrm[:ns, blk], ptr[:ns, blk])
                    nc.sync.dma_start(
                        dstt[d0 + si * 128 : d0 + si * 128 + ns, :],
                        xrm[:ns, blk])

            # ---------- prologue: x = relu(x @ W_in + b_in) ----------
            for gi, grp in enumerate(GROUPS_B):
                n0 = grp[0] * 128
                nn = min(512, NO - n0)
                nsc = (nn + 127) // 128
                xin = wp.tile([128, 512], f32, tag="xin")
                for si in range(nsc):
                    ns = min(128, nn - si * 128)
                    nc.sync.dma_start(xin[:ns, si * 128 : si * 128 + C],
                                      xsh[n0 + si * 128 : n0 + si * 128 + ns, :])
                ptr = psD.tile([128, 512], f32, tag="tr")
                for si in range(nsc):
                    ns = min(128, nn - si * 128)
                    nc.tensor.transpose(ptr[:, si * 128 : si * 128 + ns],
                                        xin[:ns, si * 128 : si * 128 + C],
                                        IDENT[:ns, :ns])
                xT = wp.tile([C, 512], f32, tag="xT")
                nc.vector.tensor_copy(xT[:, :nn], ptr[:, :nn])
                pmm = psB.tile([C, 512], f32, tag="mmA")
                nc.tensor.matmul(pmm[:, :nn], W_IN[:], xT[:, :nn])
                nc.vector.tensor_scalar(XFM[:, n0 : n0 + nn], pmm[:, :nn],
                                        BIN[:, :1], 0.0, mybir.AluOpType.add,
                                        mybir.AluOpType.max)
                # X0B = x0 + degV*b2 + (W2b^T b1)*s2   (host-built addend)
                dvt = wp.tile([C, 512], f32, tag="dvt")
                nc.sync.dma_start(dvt[:, :nn], x0add_d[:, n0 : n0 + nn])
                nc.vector.tensor_tensor(X0B[:, n0 : n0 + nn], dvt[:, :nn],
                                        XFM[:, n0 : n0 + nn],
                                        mybir.AluOpType.add)
                # row-major f16 copy (local phase-A gather source)
                ptr2 = psD.tile([128, 512], f32, tag="tr")
                xrm = wp.tile([128, 512], f16, tag="xrm")
                emit_rm(XFM, n0, nn, ptr2, xrm, agx[0], n0)

            # ---------- conv layers ----------
            for l in range(N_LAYERS):
                # ---- phase A: partial Xe' = (segsum_E x[V_local]) @ W1 over
                # ALL edges, then AllReduce(add) ----
                colA = 0
                slotA = 0
                for grp in GROUPS_P:
                    ntot = sum(nchP[t] for t in grp)
                    gt = wp.tile([128, ntot, C], f16, tag="gath")
                    split_gather(gt, 0, ntot, agx[l][:], IDXA, slotA)
                    slotA += ntot * 128
                    Pg = pgen(RELA, colA, ntot)
                    # quads of 4 SCs share one [C,512] psum + W1 matmul
                    off = 0
                    for q0 in range(0, len(grp), 4):
                        quad = grp[q0 : q0 + 4]
                        e0q = quad[0] * 128
                        ne_q = min(128 * len(quad), M - e0q)
                        ps = psA.tile([C, 512], f32, tag="seg")
                        for si, t in enumerate(quad):
                            dst = ps[:, si * 128 : (si + 1) * 128]
                            nchs = nchP[t]
                            for j in range(nchs):
                                nc.tensor.matmul(dst, gt[:, off + j, :],
                                                 Pg[:, off + j, :],
                                                 start=(j == 0),
                                                 stop=(j == nchs - 1))
                            off += nchs
                        gsb = wp.tile([C, 512], f32, tag="gsb")
                        nc.vector.tensor_copy(gsb[:, :ne_q], ps[:, :ne_q])
                        pxe = psB.tile([C, 512], f32, tag="mmA")
                        nc.tensor.matmul(pxe[:, :ne_q], W1[:], gsb[:, :ne_q])
                        xesb = wp.tile([C, 512], f32, tag="xesb")
                        nc.vector.tensor_copy(xesb[:, :ne_q], pxe[:, :ne_q])
                        ptr = psD.tile([128, 512], f32, tag="tr")
                        xerm = wp.tile([128, 512], f16, tag="xrm")
                        emit_rm(xesb, 0, ne_q, ptr, xerm, xep[l], e0q)
                    colA += ntot
                nc.gpsimd.collective_compute(
                    "AllReduce", mybir.AluOpType.add, replica_groups=rg,
                    ins=[xep[l][:]], outs=[xe_full[l][:]],
                )

                # ---- phase B ----
                last = l == N_LAYERS - 1
                colB = 0
                slotB = 0
                for gi, grp in enumerate(GROUPS_B):
                    n0g = grp[0] * 128
                    nn_g = min(128 * len(grp), NO - n0g)
                    ntot = sum(nchB[s] for s in grp)
                    gt = wp.tile([128, ntot, C], f16, tag="gath")
                    split_gather(gt, 0, ntot, xe_full[l][:], IDXB, slotB)
                    slotB += ntot * 128
                    Pg = pgen(RELB, colB, ntot)
                    ps = psA.tile([C, 512], f32, tag="seg")
                    off = 0
                    for si, s in enumerate(grp):
                        dst = ps[:, si * 128 : (si + 1) * 128]
                        nchs = nchB[s]
                        for j in range(nchs):
                            nc.tensor.matmul(dst, gt[:, off + j, :],
                                             Pg[:, off + j, :],
                                             start=(j == 0),
                                             stop=(j == nchs - 1))
                        off += nchs
                    colB += ntot
                    ysb = wp.tile([C, 512], f32, tag="gsb")
                    nc.vector.tensor_copy(ysb[:, :nn_g], ps[:, :nn_g])
                    dvt = wp.tile([C, 512], f32, tag="dvt")
                    nc.sync.dma_start(dvt[:, :nn_g], dvrep_d[:, n0g : n0g + nn_g])
                    xdeg = wp.tile([C, 512], f32, tag="xdeg")
                    nc.vector.tensor_tensor(xdeg[:, :nn_g],
                                            XFM[:, n0g : n0g + nn_g],
                                            dvt[:, :nn_g],
                                            mybir.AluOpType.mult)
                    pab = psB.tile([C, 512], f32, tag="mmA")
                    nc.tensor.matmul(pab[:, :nn_g], W2A[:], xdeg[:, :nn_g],
                                     start=True, stop=False)
                    nc.tensor.matmul(pab[:, :nn_g], W2B[:], ysb[:, :nn_g],
                                     start=False, stop=True)
                    xmid = wp.tile([C, 512], f32, tag="xesb")
                    nc.vector.tensor_tensor(xmid[:, :nn_g], pab[:, :nn_g],
                                            X0B[:, n0g : n0g + nn_g],
                                            mybir.AluOpType.add)
                    pc = psC.tile([C, 512], f32, tag="out")
                    nc.tensor.matmul(pc[:, :nn_g], W3H[:], xmid[:, :nn_g])
                    nc.vector.tensor_scalar(XFM[:, n0g : n0g + nn_g],
                                            pc[:, :nn_g], B3[:, :1], 0.0,
                                            mybir.AluOpType.add,
                                            mybir.AluOpType.max)
                    ptr = psD.tile([128, 512], f32, tag="tr")
                    xrm = wp.tile([128, 512], f32 if last else f16,
                                  tag="xrmf" if last else "xrm")
                    dstt = xout if last else agx[l + 1]
                    emit_rm(XFM, n0g, nn_g, ptr, xrm, dstt, n0g)
    nc.compile()
    return nc


def _get_program(V, E):
    key = (hash(V.tobytes()), hash(E.tobytes()))
    if key not in _cache:
        meta, per_core = _prepare(V, E)
        nc = _build(meta)
        _cache[key] = (nc, per_core)
    return _cache[key]


def run(trace=False, trace_kwargs=None, **inputs):
    x = np.ascontiguousarray(np.asarray(inputs["x"], dtype=np.float32))
    V = np.asarray(inputs["V"]).astype(np.int64)
    E = np.asarray(inputs["E"]).astype(np.int64)
    W_in = np.ascontiguousarray(np.asarray(inputs["W_in"], np.float32))
    b_in = np.asarray(inputs["b_in"], np.float32).reshape(C, 1)
    W1 = np.ascontiguousarray(np.asarray(inputs["W1"], np.float32))
    b1 = np.asarray(inputs["b1"], np.float32).reshape(C)
    W2 = np.asarray(inputs["W2"], np.float32)
    b2 = np.asarray(inputs["b2"], np.float32).reshape(C)
    W3 = np.asarray(inputs["W3"], np.float32)
    b3 = np.asarray(inputs["b3"], np.float32).reshape(C, 1)
    W2a = np.ascontiguousarray(W2[:C])
    W2b = np.ascontiguousarray(W2[C:])
    W3h = np.ascontiguousarray((1.0 - ALPHA) * W3)
    # note: (1-a)*Xv + a*x0 = (1-a)*(Xv + x0) since a = 0.5
    corr = W2b.T @ b1   # phase-B compensation for the folded b1*degE bias

    nc, per_core = _get_program(V, E)

    in_maps = []
    for r in range(R):
        pc = per_core[r]
        x0add = np.ascontiguousarray(
            (np.outer(b2, pc["degV"]) + np.outer(corr, pc["s2"]))
            .astype(np.float32))
        dvrep = np.ascontiguousarray(
            np.broadcast_to(pc["degV"], (C, NO)).astype(np.float32))
        in_maps.append({
            "xsh": x[r * NO : (r + 1) * NO],
            "w_in": W_in, "w1": W1, "w2a": W2a, "w2b": W2b, "w3h": W3h,
            "b_in": b_in, "b3": b3,
            "x0add": x0add, "dvrep": dvrep,
            "idxA": pc["idxA"], "relA": pc["relA"],
            "idxB": pc["idxB"], "relB": pc["relB"],
        })
    res = run_bass_kernel_spmd(nc, in_maps, list(range(R)), trace=trace,
                               **(trace_kwargs or {}))
    out = np.concatenate([res.results[r]["xout"] for r in range(R)], axis=0)
    return out, res


def kernel(**inputs):
    out, _ = run(**inputs)
    return out


# revision 21
# speedup vs baseline: 1.4515x; 1.4515x over previous
"""Bass/Trainium2 kernel for nn_EquivSetGNN3 (gnn_message_passing).

Math (reference): x = relu(x@W_in+b_in); x0 = x
  2 layers of: Xe = segsum_E((x@W1+b1)[V]); Xev = cat(x[V], Xe[E])@W2+b2
               Xv = segsum_V(Xev); x = relu((0.5*Xv + 0.5*x0)@W3 + b3)

Algebraic restructuring (avoids all [nnz, C] feature materialization):
  Xe = (segsum_E x[V]) @ W1 + deg_E (x) b1
  Xv = deg_V (*) (x @ W2a) + (segsum_V Xe[E]) @ W2b + deg_V (x) b2
where W2a = W2[:C], W2b = W2[C:].

Segment sums run as dma_gather of 256B f16 rows + one-hot matmuls on the
TensorEngine. Gathers are batched into grouped-superchunk calls split at
~2048 idxs across 4 SWDGE queues; the one-hot P matrices are generated on
the fly by the DVE (is_equal vs an iota row, broadcast APs).

Pipelined buckets: x is split into thirds and Xe into a 80/20 pair of
bucket tensors at group boundaries. Each bucket's AllGather fires as soon
as its producing groups finish (overlapping the rest of the phase), and
the next phase's gather indices are bucketed accordingly, so the in-order
gpsimd queue stalls only on the last (small, early-ish) collective. The
buckets also keep all gather indices < 32768 (int16).
"""
import numpy as np

import concourse.bacc as bacc
import concourse.bass as bass_mod
import concourse.mybir as mybir
import concourse.tile as tile
from concourse.bass_utils import run_bass_kernel_spmd

f32 = mybir.dt.float32
f16 = mybir.dt.float16
i16 = mybir.dt.int16

N = 50000
M = 25000
NNZ = 800000
C = 128
R = 8
NO = N // R          # 6250 nodes per core
EO = M // R          # 3125 edges per core
SCA = (EO + 127) // 128   # 25 edge superchunks per core
SCB = (NO + 127) // 128   # 49 node superchunks per core
GA = 2               # phase-A superchunks per gather group
GB = 4               # phase-B (and prologue) superchunks per group
N_LAYERS = 2
ALPHA = 0.5
SPLIT_CHUNKS = 16    # max chunks (128 idxs each) per dma_gather call

GROUPS_A = [list(range(g, min(g + GA, SCA))) for g in range(0, SCA, GA)]
GROUPS_B = [list(range(g, min(g + GB, SCB))) for g in range(0, SCB, GB)]
# bucket cuts: after these group indices, fire the bucket's AllGather.
# node buckets (phase B / prologue groups of GB*128=512 rows)
NCUT_GROUPS = [6, len(GROUPS_B) - 1]
NBOUNDS = [0, 3584, NO]
# edge buckets (phase A groups of GA*128=256 rows)
ECUT_GROUPS = [len(GROUPS_A) - 1]
EBOUNDS = [0, EO]
NSZ = [NBOUNDS[k + 1] - NBOUNDS[k] for k in range(len(NBOUNDS) - 1)]
ESZ = [EBOUNDS[k + 1] - EBOUNDS[k] for k in range(len(EBOUNDS) - 1)]
assert all(R * s < 32768 for s in NSZ + ESZ)
NBK = len(NSZ)
EBK = len(ESZ)

_cache = {}


def _wrap_idx(flat):
    """[L] int -> [128, L//16] int16 (idx i at partition i%16, col i//16;
    replicated 8x across partition groups for the 8 gpsimd cores)."""
    w = flat.reshape(-1, 16).T.astype(np.int16)
    return np.ascontiguousarray(np.tile(w, (8, 1)))


def _wrap_rel(flat):
    """[L] int -> [128, L//128] f16 (value i at partition i%128, col i//128)."""
    return np.ascontiguousarray(flat.reshape(-1, 128).T.astype(np.float16))


def _pad_to(arr, n, val):
    out = np.full(n, val, dtype=arr.dtype)
    out[: len(arr)] = arr
    return out


def _bucketize(key_vals, rel_vals, windows, groups, per, bounds, sizes, r,
               nch):
    """Build per-group [bk0 chunks of each SC][bk1 ...] idx/rel streams.

    key_vals: global ids being gathered; per: rows per core in the source
    (NO or EO); bounds/sizes: bucket bounds in per-core row offset.
    """
    idx_parts, rel_parts = [], []
    for grp in groups:
        segs = {}
        for s in grp:
            lo, hi = windows[r][s]
            segs[s] = (key_vals[lo:hi], rel_vals[lo:hi])
        for k in range(len(sizes)):
            for s in grp:
                key, rel = segs[s]
                j = key % per
                m = (j >= bounds[k]) & (j < bounds[k + 1])
                idx = (key[m] // per) * sizes[k] + (j[m] - bounds[k])
                la = int(nch[k][s]) * 128
                idx_parts.append(_pad_to(idx, la, 0))
                rel_parts.append(_pad_to(rel[m], la, -1))
    return np.concatenate(idx_parts), np.concatenate(rel_parts)


def _prepare(V, E):
    """Host-side preprocessing: sorted/sharded/padded gather index+rel
    arrays, bucketed by gather-source bucket (see module docstring)."""
    # ---- phase A: incidences sorted by E (edge-major); gathers x[V] ----
    oA = np.argsort(E, kind="stable")
    Va, Ea = V[oA], E[oA]
    e0 = np.arange(R)[:, None] * EO + np.arange(SCA)[None, :] * 128
    e1 = np.minimum(e0 + 128, (np.arange(R)[:, None] + 1) * EO)
    lo = np.searchsorted(Ea, e0.ravel()).reshape(R, SCA)
    hi = np.searchsorted(Ea, e1.ravel()).reshape(R, SCA)

    vj = Va % NO
    nchA = []
    for k in range(NBK):
        mk = (vj >= NBOUNDS[k]) & (vj < NBOUNDS[k + 1])
        cnt = np.zeros((R, SCA), np.int64)
        for r in range(R):
            for s in range(SCA):
                cnt[r, s] = int(mk[lo[r, s] : hi[r, s]].sum())
        nchA.append((-(-cnt.max(0) // 128)).astype(np.int64))

    # ---- phase B: incidences sorted by V (node-major); gathers Xe[E] ----
    oB = np.argsort(V, kind="stable")
    Vb, Eb = V[oB], E[oB]
    v0 = np.arange(R)[:, None] * NO + np.arange(SCB)[None, :] * 128
    v1 = np.minimum(v0 + 128, (np.arange(R)[:, None] + 1) * NO)
    lo2 = np.searchsorted(Vb, v0.ravel()).reshape(R, SCB)
    hi2 = np.searchsorted(Vb, v1.ravel()).reshape(R, SCB)

    ej = Eb % EO
    nchB = []
    for k in range(EBK):
        mk = (ej >= EBOUNDS[k]) & (ej < EBOUNDS[k + 1])
        cnt = np.zeros((R, SCB), np.int64)
        for r in range(R):
            for s in range(SCB):
                cnt[r, s] = int(mk[lo2[r, s] : hi2[r, s]].sum())
        nchB.append((-(-cnt.max(0) // 128)).astype(np.int64))

    meta = {
        "nchA": [a.tolist() for a in nchA],
        "nchB": [b.tolist() for b in nchB],
    }

    degE = np.bincount(E, minlength=M).astype(np.float32)
    degV = np.bincount(V, minlength=N).astype(np.float32)

    winA = [[(lo[r, s], hi[r, s]) for s in range(SCA)] for r in range(R)]
    winB = [[(lo2[r, s], hi2[r, s]) for s in range(SCB)] for r in range(R)]
    relEa = Ea % EO % 128 + (Ea % EO // 128 * 128) - (Ea % EO // 128 * 128)
    relEa = Ea % EO - (Ea % EO // 128 * 128)   # edge offset within SC
    relVb = Vb % NO - (Vb % NO // 128 * 128)   # node offset within SC

    per_core = []
    for r in range(R):
        idxA, relA = _bucketize(Va, relEa, winA, GROUPS_A, NO, NBOUNDS, NSZ,
                                r, nchA)
        idxB, relB = _bucketize(Eb, relVb, winB, GROUPS_B, EO, EBOUNDS, ESZ,
                                r, nchB)
        per_core.append(
            {
                "idxA": _wrap_idx(idxA),
                "relA": _wrap_rel(relA),
                "idxB": _wrap_idx(idxB),
                "relB": _wrap_rel(relB),
                "degE": degE[r * EO : (r + 1) * EO],
                "degV": degV[r * NO : (r + 1) * NO],
            }
        )
    return meta, per_core


def _build(meta):
    nchA = meta["nchA"]   # [NBK][SCA]
    nchB = meta["nchB"]   # [EBK][SCB]
    LA = sum(sum(a) for a in nchA) * 128
    LB = sum(sum(b) for b in nchB) * 128
    NCHA_TOT = LA // 128
    NCHB_TOT = LB // 128

    nc = bacc.Bacc("TRN2", target_bir_lowering=False, debug=False, num_devices=R,
                   num_swdge_queues=4)

    # ---- kernel I/O ----
    xsh = nc.declare_dram_parameter("xsh", [NO, C], f32, isOutput=False)
    w_in = nc.declare_dram_parameter("w_in", [C, C], f32, isOutput=False)
    w1 = nc.declare_dram_parameter("w1", [C, C], f32, isOutput=False)
    w2a = nc.declare_dram_parameter("w2a", [C, C], f32, isOutput=False)
    w2b = nc.declare_dram_parameter("w2b", [C, C], f32, isOutput=False)
    w3h = nc.declare_dram_parameter("w3h", [C, C], f32, isOutput=False)
    b_in = nc.declare_dram_parameter("b_in", [C, 1], f32, isOutput=False)
    b2d = nc.declare_dram_parameter("b2", [C, 1], f32, isOutput=False)
    b3d = nc.declare_dram_parameter("b3", [C, 1], f32, isOutput=False)
    b1e_d = nc.declare_dram_parameter("b1e", [C, EO], f32, isOutput=False)
    dvrep_d = nc.declare_dram_parameter("dvrep", [C, NO], f32, isOutput=False)
    idxA_d = nc.declare_dram_parameter("idxA", [128, LA // 16], i16, isOutput=False)
    relA_d = nc.declare_dram_parameter("relA", [128, NCHA_TOT], f16, isOutput=False)
    idxB_d = nc.declare_dram_parameter("idxB", [128, LB // 16], i16, isOutput=False)
    relB_d = nc.declare_dram_parameter("relB", [128, NCHB_TOT], f16, isOutput=False)
    xout = nc.declare_dram_parameter("xout", [NO, C], f32, isOutput=True)

    # ---- internal DRAM (bucketed gather sources) ----
    agx = [[nc.dram_tensor(f"agx{k}_{l}", [NSZ[k], C], f16)
            for k in range(NBK)] for l in range(N_LAYERS)]
    x_bk = [[nc.dram_tensor(f"x_bk{k}_{l}", [R * NSZ[k], C], f16,
                            addr_space="Shared")
             for k in range(NBK)] for l in range(N_LAYERS)]
    agxe = [[nc.dram_tensor(f"agxe{k}_{l}", [ESZ[k], C], f16)
             for k in range(EBK)] for l in range(N_LAYERS)]
    xe_bk = [[nc.dram_tensor(f"xe_bk{k}_{l}", [R * ESZ[k], C], f16,
                             addr_space="Shared")
              for k in range(EBK)] for l in range(N_LAYERS)]

    rg = [list(range(R))]
    qrr = [0]

    def next_q():
        q = qrr[0]
        qrr[0] = (q + 1) % 4
        return q

    def ag(t_in, t_out):
        nc.gpsimd.collective_compute( "AllGather", mybir.AluOpType.bypass, replica_groups=rg,
            ins=[t_in[:]], outs=[t_out[:]],
        )

    def out_bucket(gi, cuts):
        for k, cg in enumerate(cuts):
            if gi <= cg:
                return k
        raise AssertionError

    with tile.TileContext(nc) as tc:
        with (
            tc.tile_pool(name="const", bufs=1) as cp,
            tc.tile_pool(name="work", bufs=2) as wp,
            tc.tile_pool(name="ptiles", bufs=3) as pp,
            tc.tile_pool(name="psA", bufs=2, space="PSUM") as psA,
            tc.tile_pool(name="psB", bufs=2, space="PSUM") as psB,
            tc.tile_pool(name="psC", bufs=2, space="PSUM") as psC,
            tc.tile_pool(name="psD", bufs=2, space="PSUM") as psD,
        ):
            # ---------- persistent tiles ----------
            W_IN = cp.tile([C, C], f32)
            W1 = cp.tile([C, C], f32)
            W2A = cp.tile([C, C], f32)
            W2B = cp.tile([C, C], f32)
            W3H = cp.tile([C, C], f32)
            BIN = cp.tile([C, 1], f32)
            B2 = cp.tile([C, 1], f32)
            B3 = cp.tile([C, 1], f32)
            IDXA = cp.tile([128, LA // 16], i16)
            IDXB = cp.tile([128, LB // 16], i16)
            RELA = cp.tile([128, NCHA_TOT], f16)
            RELB = cp.tile([128, NCHB_TOT], f16)
            XFM = cp.tile([C, NO], f32)
            X0B = cp.tile([C, NO], f32)
            IOTAF = cp.tile([128, 128], f32)
            IOTA16 = cp.tile([128, 128], f16)
            PIDX = cp.tile([128, 1], f32)
            IDENT = cp.tile([128, 128], f32)

            for t, d in [
                (W_IN, w_in), (W1, w1), (W2A, w2a), (W2B, w2b), (W3H, w3h),
                (BIN, b_in), (B2, b2d), (B3, b3d),
                (IDXA, idxA_d), (IDXB, idxB_d), (RELA, relA_d), (RELB, relB_d),
            ]:
                nc.sync.dma_start(t[:], d[:])

            nc.gpsimd.iota(IOTAF[:], [[1, 128]], channel_multiplier=0,
                           allow_small_or_imprecise_dtypes=True)
            nc.gpsimd.iota(PIDX[:], [[1, 1]], channel_multiplier=1,
                           allow_small_or_imprecise_dtypes=True)
            nc.vector.tensor_scalar(IDENT[:], IOTAF[:], PIDX[:], None,
                                    mybir.AluOpType.is_equal)
            nc.vector.tensor_copy(IOTA16[:], IOTAF[:])

            def split_gather(gt, col0, nch_tot, src_ap, idx_tile, slot0):
                """One logical gather as ceil(nch_tot/SPLIT_CHUNKS) calls on
                rotating SWDGE queues (parallel descriptor gen)."""
                done = 0
                while done < nch_tot:
                    step = min(SPLIT_CHUNKS, nch_tot - done)
                    sl = slot0 + done * 128
                    nc.gpsimd.dma_gather(
                        out_ap=gt[:, col0 + done : col0 + done + step, :],
                        in_ap=src_ap,
                        idxs_ap=idx_tile[:, sl // 16 : (sl + step * 128) // 16],
                        num_idxs=step * 128, num_idxs_reg=step * 128,
                        elem_size=C, single_packet=False, queue_num=next_q(),
                    )
                    done += step

            def pgen(rel_tile, c0, nch):
                """P[p, c, j] = (rel[p, c0+c] == j), one DVE op."""
                P = pp.tile([128, max(nch, 1), 128], f16, tag="P")
                nc.vector.tensor_tensor(
                    P[:, :nch, :],
                    rel_tile[:, c0 : c0 + nch].unsqueeze(2)
                        .broadcast_to([128, nch, 128]),
                    IOTA16[:].unsqueeze(1).broadcast_to([128, nch, 128]),
                    mybir.AluOpType.is_equal)
                return P

            def gather_group(grp, nch_by_bk, src_list, idx_tile, slot0):
                """Bucketed gathers for one group into a fresh gt tile.
                Returns (gt, ntot, per-SC chunk-col seqs)."""
                bk_tot = [sum(nch[s] for s in grp) for nch in nch_by_bk]
                ntot = sum(bk_tot)
                gt = wp.tile([128, ntot, C], f16, tag="gath")
                col = 0
                sl = slot0
                for k, nk in enumerate(bk_tot):
                    if nk > 0:
                        split_gather(gt, col, nk, src_list[k][:], idx_tile, sl)
                    col += nk
                    sl += nk * 128
                seqs = {s: [] for s in grp}
                off = 0
                for k, nch in enumerate(nch_by_bk):
                    for s in grp:
                        seqs[s].extend(range(off, off + nch[s]))
                        off += nch[s]
                return gt, ntot, seqs

            def seg_matmuls(grp, gt, Pg, seqs, ps):
                for si, s in enumerate(grp):
                    seq = seqs[s]
                    dst = ps[:, si * 128 : (si + 1) * 128]
                    for j, cc in enumerate(seq):
                        nc.tensor.matmul(dst, gt[:, cc, :], Pg[:, cc, :],
                                         start=(j == 0),
                                         stop=(j == len(seq) - 1))

            def emit_rm(src_fm, n0, nn, ptr, xrm, dstt, base):
                """Transpose feature-major [C, nn] slice to row-major f16/f32
                blocks and DMA to DRAM rows [n0-base, n0-base+nn)."""
                for si in range((nn + 127) // 128):
                    ns = min(128, nn - si * 128)
                    blk = slice(si * 128, si * 128 + C)
                    nc.tensor.transpose(
                        ptr[:ns, blk],
                        src_fm[:, n0 + si * 128 : n0 + si * 128 + ns],
                        IDENT[:])
                    nc.vector.tensor_copy(xrm[:ns, blk], ptr[:ns, blk])
                    r0 = n0 - base + si * 128
                    nc.sync.dma_start(dstt[r0 : r0 + ns, :], xrm[:ns, blk])

            # ---------- prologue: x = relu(x @ W_in + b_in) ----------
            for gi, grp in enumerate(GROUPS_B):
                n0 = grp[0] * 128
                nn = min(512, NO - n0)
                nsc = (nn + 127) // 128
                xin = wp.tile([128, 512], f32, tag="xin")
                for si in range(nsc):
                    ns = min(128, nn - si * 128)
                    nc.sync.dma_start(xin[:ns, si * 128 : si * 128 + C],
                                      xsh[n0 + si * 128 : n0 + si * 128 + ns, :])
                ptr = psD.tile([128, 512], f32, tag="tr")
                for si in range(nsc):
                    ns = min(128, nn - si * 128)
                    nc.tensor.transpose(ptr[:, si * 128 : si * 128 + ns],
                                        xin[:ns, si * 128 : si * 128 + C],
                                        IDENT[:ns, :ns])
                xT = wp.tile([C, 512], f32, tag="xT")
                nc.vector.tensor_copy(xT[:, :nn], ptr[:, :nn])
                pmm = psB.tile([C, 512], f32, tag="mmA")
                nc.tensor.matmul(pmm[:, :nn], W_IN[:], xT[:, :nn])
                nc.vector.tensor_scalar(XFM[:, n0 : n0 + nn], pmm[:, :nn],
                                        BIN[:, :1], 0.0, mybir.AluOpType.add,
                                        mybir.AluOpType.max)
                # X0B = x0 + deg_V * b2  (phase-B restart + bias, pre-folded)
                dvt = wp.tile([C, 512], f32, tag="dvt")
                nc.sync.dma_start(dvt[:, :nn], dvrep_d[:, n0 : n0 + nn])
                tmp = wp.tile([C, 512], f32, tag="xdeg")
                nc.vector.tensor_scalar(tmp[:, :nn], dvt[:, :nn], B2[:, :1],
                                        None, mybir.AluOpType.mult)
                nc.vector.tensor_tensor(X0B[:, n0 : n0 + nn], tmp[:, :nn],
                                        XFM[:, n0 : n0 + nn],
                                        mybir.AluOpType.add)
                # row-major f16 copy (gather source)
                k = out_bucket(gi, NCUT_GROUPS)
                ptr2 = psD.tile([128, 512], f32, tag="tr")
                xrm = wp.tile([128, 512], f16, tag="xrm")
                emit_rm(XFM, n0, nn, ptr2, xrm, agx[0][k], NBOUNDS[k])
                if gi in NCUT_GROUPS:
                    k = NCUT_GROUPS.index(gi)
                    ag(agx[0][k], x_bk[0][k])

            # ---------- conv layers ----------
            for l in range(N_LAYERS):
                # ---- phase A: Xe = (segsum_E x[V]) @ W1 + b1*degE ----
                colA = 0
                slotA = 0
                for gi, grp in enumerate(GROUPS_A):
                    e0g = grp[0] * 128
                    ne_g = min(128 * len(grp), EO - e0g)
                    gt, ntot, seqs = gather_group(grp, nchA, x_bk[l], IDXA,
                                                  slotA)
                    slotA += ntot * 128
                    Pg = pgen(RELA, colA, ntot)
                    colA += ntot
                    ps = psA.tile([C, 512], f32, tag="seg")
                    seg_matmuls(grp, gt, Pg, seqs, ps)
                    gsb = wp.tile([C, 512], f32, tag="gsb")
                    nc.vector.tensor_copy(gsb[:, :ne_g], ps[:, :ne_g])
                    b1t = wp.tile([C, 512], f32, tag="dvt")
                    nc.sync.dma_start(b1t[:, :ne_g], b1e_d[:, e0g : e0g + ne_g])
                    pxe = psB.tile([C, 512], f32, tag="mmA")
                    nc.tensor.matmul(pxe[:, :ne_g], W1[:], gsb[:, :ne_g])
                    xesb = wp.tile([C, 512], f32, tag="xesb")
                    nc.vector.tensor_tensor(xesb[:, :ne_g], pxe[:, :ne_g],
                                            b1t[:, :ne_g], mybir.AluOpType.add)
                    k = out_bucket(gi, ECUT_GROUPS)
                    ptr = psD.tile([128, 512], f32, tag="tr")
                    xerm = wp.tile([128, 512], f16, tag="xrm")
                    emit_rm(xesb, 0, ne_g, ptr, xerm, agxe[l][k],
                            EBOUNDS[k] - e0g)
                    if gi in ECUT_GROUPS:
                        k = ECUT_GROUPS.index(gi)
                        ag(agxe[l][k], xe_bk[l][k])

                # ---- phase B ----
                last = l == N_LAYERS - 1
                colB = 0
                slotB = 0
                for gi, grp in enumerate(GROUPS_B):
                    n0g = grp[0] * 128
                    nn_g = min(128 * len(grp), NO - n0g)
                    gt, ntot, seqs = gather_group(grp, nchB, xe_bk[l], IDXB,
                                                  slotB)
                    slotB += ntot * 128
                    Pg = pgen(RELB, colB, ntot)
                    colB += ntot
                    ps = psA.tile([C, 512], f32, tag="seg")
                    seg_matmuls(grp, gt, Pg, seqs, ps)
                    ysb = wp.tile([C, 512], f32, tag="gsb")
                    nc.vector.tensor_copy(ysb[:, :nn_g], ps[:, :nn_g])
                    dvt = wp.tile([C, 512], f32, tag="dvt")
                    nc.sync.dma_start(dvt[:, :nn_g], dvrep_d[:, n0g : n0g + nn_g])
                    xdeg = wp.tile([C, 512], f32, tag="xdeg")
                    nc.vector.tensor_tensor(xdeg[:, :nn_g],
                                            XFM[:, n0g : n0g + nn_g],
                                            dvt[:, :nn_g],
                                            mybir.AluOpType.mult)
                    pab = psB.tile([C, 512], f32, tag="mmA")
                    nc.tensor.matmul(pab[:, :nn_g], W2A[:], xdeg[:, :nn_g],
                                     start=True, stop=False)
                    nc.tensor.matmul(pab[:, :nn_g], W2B[:], ysb[:, :nn_g],
                                     start=False, stop=True)
                    xmid = wp.tile([C, 512], f32, tag="xesb")
                    nc.vector.tensor_tensor(xmid[:, :nn_g], pab[:, :nn_g],
                                            X0B[:, n0g : n0g + nn_g],
                                            mybir.AluOpType.add)
                    pc = psC.tile([C, 512], f32, tag="out")
                    nc.tensor.matmul(pc[:, :nn_g], W3H[:], xmid[:, :nn_g])
                    nc.vector.tensor_scalar(XFM[:, n0g : n0g + nn_g],
                                            pc[:, :nn_g], B3[:, :1], 0.0,
                                            mybir.AluOpType.add,
                                            mybir.AluOpType.max)
                    ptr = psD.tile([128, 512], f32, tag="tr")
                    xrm = wp.tile([128, 512], f32 if last else f16,
                                  tag="xrmf" if last else "xrm")
                    if last:
                        emit_rm(XFM, n0g, nn_g, ptr, xrm, xout, 0)
                    else:
                        k = out_bucket(gi, NCUT_GROUPS)
                        emit_rm(XFM, n0g, nn_g, ptr, xrm, agx[l + 1][k],
                                NBOUNDS[k])
                        if gi in NCUT_GROUPS:
                            k = NCUT_GROUPS.index(gi)
                            ag(agx[l + 1][k], x_bk[l + 1][k])
    nc.compile()
    return nc


def _get_program(V, E):
    key = (hash(V.tobytes()), hash(E.tobytes()))
    if key not in _cache:
        meta, per_core = _prepare(V, E)
        nc = _build(meta)
        _cache[key] = (nc, per_core)
    return _cache[key]


def run(trace=False, trace_kwargs=None, **inputs):
    x = np.ascontiguousarray(np.asarray(inputs["x"], dtype=np.float32))
    V = np.asarray(inputs["V"]).astype(np.int64)
    E = np.asarray(inputs["E"]).astype(np.int64)
    W_in = np.ascontiguousarray(np.asarray(inputs["W_in"], np.float32))
    b_in = np.asarray(inputs["b_in"], np.float32).reshape(C, 1)
    W1 = np.ascontiguousarray(np.asarray(inputs["W1"], np.float32))
    b1 = np.asarray(inputs["b1"], np.float32).reshape(C)
    W2 = np.asarray(inputs["W2"], np.float32)
    b2 = np.asarray(inputs["b2"], np.float32).reshape(C, 1)
    W3 = np.asarray(inputs["W3"], np.float32)
    b3 = np.asarray(inputs["b3"], np.float32).reshape(C, 1)
    W2a = np.ascontiguousarray(W2[:C])
    W2b = np.ascontiguousarray(W2[C:])
    W3h = np.ascontiguousarray((1.0 - ALPHA) * W3)
    # note: (1-a)*Xv + a*x0 = (1-a)*(Xv + x0) since a = 0.5

    nc, per_core = _get_program(V, E)

    in_maps = []
    for r in range(R):
        pc = per_core[r]
        b1e = np.ascontiguousarray(np.outer(b1, pc["degE"]).astype(np.float32))
        dvrep = np.ascontiguousarray(
            np.broadcast_to(pc["degV"], (C, NO)).astype(np.float32))
        in_maps.append({
            "xsh": x[r * NO : (r + 1) * NO],
            "w_in": W_in, "w1": W1, "w2a": W2a, "w2b": W2b, "w3h": W3h,
            "b_in": b_in, "b2": b2, "b3": b3,
            "b1e": b1e, "dvrep": dvrep,
            "idxA": pc["idxA"], "relA": pc["relA"],
            "idxB": pc["idxB"], "relB": pc["relB"],
        })
    res = run_bass_kernel_spmd(nc, in_maps, list(range(R)), trace=trace,
                               **(trace_kwargs or {}))
    out = np.concatenate([res.results[r]["xout"] for r in range(R)], axis=0)
    return out, res


def kernel(**inputs):
    out, _ = run(**inputs)
    return out
